# revision 6
# baseline (speedup 1.0000x reference)
"""Squared-exponential kernel expansion on 8 Trainium2 NeuronCores.

Computes out[m] = sigma^2 * sum_n w_n * exp(-||s_n - x_m||^2 / (2 l^2)),
x: [16384, 24], samples: [8192, 24], weights: [8192] -> out [16384, 1].

Strategy: shard queries (M) across 8 cores. Algebraic folding puts the whole
exponent into one augmented matmul:
    z[m, n] = 2g*x_m.s_n + (ln(w_n sigma^2) - g||s_n||^2) + (-g||x_m||^2)
with g = 0.5/length^2, so out[m] = sum_n exp(z[m, n]).
The fp32 dot products are emulated with bf16 hi/lo splits packed into the
contraction dim (K = 3*24 cross rows + 3 b-rows + 3 c-rows = 78), one bf16
matmul per tile at full PE rate. ScalarE evaluates exp over each [128, 2048]
PSUM span with accum_out producing the n-partial sums for free.
"""

import os
import sys

for _p in ("/opt/trn_rl_repo",):
    if _p not in sys.path and os.path.isdir(_p):
        sys.path.insert(0, _p)

import numpy as np
import ml_dtypes

BF16 = ml_dtypes.bfloat16

M_TOT = 16384
N_TOT = 8192
D = 24
N_CORES = 8
M_LOC = M_TOT // N_CORES  # 2048
K_AUG = 3 * D + 6  # 78
M_CHUNK = 128
N_SPAN = 2048  # one ScalarE activation span (4 PSUM banks)
N_SUB = 512  # one matmul free dim (1 PSUM bank)

_CACHE = {}
LAST_RESULTS = None  # BassKernelResults from the most recent run (for test.py)


def _split2(a):
    """fp32 array -> (hi, lo) bf16 with a ~= hi + lo."""
    hi = a.astype(BF16)
    lo = (a - hi.astype(np.float32)).astype(BF16)
    return hi, lo


def _split3(a):
    h = a.astype(BF16)
    r = a - h.astype(np.float32)
    m = r.astype(BF16)
    l = (r - m.astype(np.float32)).astype(BF16)
    return h, m, l


def _build_bass():
    import concourse.mybir as mybir
    import concourse.tile as tile
    from concourse import bacc

    nc = bacc.Bacc("TRN2", target_bir_lowering=False, debug=False, num_devices=N_CORES)
    aq = nc.dram_tensor("aq", [K_AUG, M_LOC], mybir.dt.bfloat16, kind="ExternalInput")
    bs = nc.dram_tensor("bs", [K_AUG, N_TOT], mybir.dt.bfloat16, kind="ExternalInput")
    y = nc.dram_tensor("y", [M_LOC, 1], mybir.dt.float32, kind="ExternalOutput")

    n_mchunks = M_LOC // M_CHUNK  # 16
    n_groups = N_TOT // N_SPAN  # 4
    n_subs = N_SPAN // N_SUB  # 4

    with tile.TileContext(nc) as tc:
        with (
            tc.tile_pool(name="inp", bufs=1) as inp,
            tc.tile_pool(name="acc", bufs=4) as accp,
            tc.tile_pool(name="escratch", bufs=2) as escratch,
            tc.tile_pool(name="out", bufs=4) as outp,
            tc.tile_pool(name="psum", bufs=2, space="PSUM") as psum,
        ):
            a_sb = inp.tile([K_AUG, M_LOC], mybir.dt.bfloat16)
            b_sb = inp.tile([K_AUG, N_TOT], mybir.dt.bfloat16)
            nc.sync.dma_start(out=a_sb, in_=aq[:, :])
            nc.sync.dma_start(out=b_sb, in_=bs[:, :])

            for mi in range(n_mchunks):
                acc = accp.tile([M_CHUNK, n_groups], mybir.dt.float32)
                lhsT = a_sb[:, mi * M_CHUNK : (mi + 1) * M_CHUNK]
                for g in range(n_groups):
                    z = psum.tile([M_CHUNK, N_SPAN], mybir.dt.float32)
                    for s in range(n_subs):
                        n0 = g * N_SPAN + s * N_SUB
                        nc.tensor.matmul(
                            z[:, s * N_SUB : (s + 1) * N_SUB],
                            lhsT=lhsT,
                            rhs=b_sb[:, n0 : n0 + N_SUB],
                            start=True,
                            stop=True,
                        )
                    e = escratch.tile([M_CHUNK, N_SPAN], mybir.dt.float32)
                    nc.scalar.activation(
                        out=e,
                        in_=z,
                        func=mybir.ActivationFunctionType.Exp,
                        accum_out=acc[:, g : g + 1],
                    )
                o = outp.tile([M_CHUNK, 1], mybir.dt.float32)
                nc.vector.reduce_sum(out=o, in_=acc, axis=mybir.AxisListType.X)
                nc.sync.dma_start(
                    out=y[mi * M_CHUNK : (mi + 1) * M_CHUNK, :], in_=o
                )
    nc.compile()
    return nc


def _prep_inputs(x, samples, weights, sigma, length):
    """Host-side O(N+M) folding: build lhsT/rhs matrices [K_AUG, *] in bf16."""
    x = np.asarray(x, dtype=np.float32)
    samples = np.asarray(samples, dtype=np.float32)
    weights = np.asarray(weights, dtype=np.float32)
    sigma = float(np.asarray(sigma))
    length = float(np.asarray(length))

    gamma = 0.5 / (length * length)
    x64 = x.astype(np.float64)
    s64 = samples.astype(np.float64)
    w64 = weights.astype(np.float64)

    # b_n = ln(w_n * sigma^2) - gamma ||s_n||^2 ; c_m = -gamma ||x_m||^2
    with np.errstate(divide="ignore"):
        b = np.log(w64 * (sigma * sigma)) - gamma * np.sum(s64 * s64, axis=1)
    b = np.where(np.isfinite(b), b, -1e30).astype(np.float32)
    c = (-gamma * np.sum(x64 * x64, axis=1)).astype(np.float32)

    xs = (2.0 * gamma) * x  # [M, D] fp32, scaled queries
    xh, xl = _split2(xs)
    sh, sl = _split2(samples)
    bh, bm, bl = _split3(b)
    ch, cm, cl = _split3(c)

    ones_m = np.ones((M_TOT,), dtype=BF16)
    ones_n = np.ones((N_TOT,), dtype=BF16)

    A = np.empty((K_AUG, M_TOT), dtype=BF16)  # lhsT side (queries)
    B = np.empty((K_AUG, N_TOT), dtype=BF16)  # rhs side (samples)
    A[0:D] = xh.T
    A[D : 2 * D] = xl.T
    A[2 * D : 3 * D] = xh.T
    B[0:D] = sh.T
    B[D : 2 * D] = sh.T
    B[2 * D : 3 * D] = sl.T
    # rows 72-74: ones (queries) x b-split (samples)
    A[72], A[73], A[74] = ones_m, ones_m, ones_m
    B[72], B[73], B[74] = bh, bm, bl
    # rows 75-77: c-split (queries) x ones (samples)
    A[75], A[76], A[77] = ch, cm, cl
    B[75], B[76], B[77] = ones_n, ones_n, ones_n

    A = np.ascontiguousarray(A)
    B = np.ascontiguousarray(B)
    return A, B


def kernel(x, samples, weights, sigma, length):
    global LAST_RESULTS
    from concourse.bass_utils import run_bass_kernel_spmd

    if "nc" not in _CACHE:
        _CACHE["nc"] = _build_bass()
    nc = _CACHE["nc"]

    A, B = _prep_inputs(x, samples, weights, sigma, length)
    in_maps = [
        {
            "aq": np.ascontiguousarray(A[:, c * M_LOC : (c + 1) * M_LOC]),
            "bs": B,
        }
        for c in range(N_CORES)
    ]
    trace = bool(os.environ.get("KERNEL_TRACE"))
    res = run_bass_kernel_spmd(
        nc,
        in_maps,
        core_ids=list(range(N_CORES)),
        trace=trace,
    )
    LAST_RESULTS = res
    out = np.concatenate([r["y"] for r in res.results], axis=0)
    return out.astype(np.float32)


# revision 9
# speedup vs baseline: 1.0329x; 1.0329x over previous
"""Squared-exponential kernel expansion on 8 Trainium2 NeuronCores.

Computes out[m] = sigma^2 * sum_n w_n * exp(-||s_n - x_m||^2 / (2 l^2)),
x: [16384, 24], samples: [8192, 24], weights: [8192] -> out [16384, 1].

Strategy: shard queries (M) across 8 cores. Algebraic folding puts the whole
exponent into one augmented matmul:
    z[m, n] = 2g*x_m.s_n + (ln(w_n sigma^2) - g||s_n||^2) + (-g||x_m||^2)
with g = 0.5/length^2, so out[m] = sum_n exp(z[m, n]).
The fp32 dot products are emulated with bf16 hi/lo splits packed into the
contraction dim (K = 3*24 cross rows + 3 b-rows + 3 c-rows = 78), one bf16
matmul per tile at full PE rate. ScalarE evaluates exp over each [128, 2048]
PSUM span with accum_out producing the n-partial sums for free.
"""

import os
import sys

for _p in ("/opt/trn_rl_repo",):
    if _p not in sys.path and os.path.isdir(_p):
        sys.path.insert(0, _p)

import numpy as np
import ml_dtypes

BF16 = ml_dtypes.bfloat16

M_TOT = 16384
N_TOT = 8192
D = 24
N_CORES = 8
M_LOC = M_TOT // N_CORES  # 2048
K_AUG = 3 * D + 6  # 78
M_CHUNK = 128
N_SPAN = 2048  # one ScalarE activation span (4 PSUM banks)
N_SUB = 512  # one matmul free dim (1 PSUM bank)

_CACHE = {}
LAST_RESULTS = None  # BassKernelResults from the most recent run (for test.py)


def _split2(a):
    """fp32 array -> (hi, lo) bf16 with a ~= hi + lo."""
    hi = a.astype(BF16)
    lo = (a - hi.astype(np.float32)).astype(BF16)
    return hi, lo


def _split3(a):
    h = a.astype(BF16)
    r = a - h.astype(np.float32)
    m = r.astype(BF16)
    l = (r - m.astype(np.float32)).astype(BF16)
    return h, m, l


def _build_bass():
    import concourse.mybir as mybir
    import concourse.tile as tile
    from concourse import bacc

    nc = bacc.Bacc("TRN2", target_bir_lowering=False, debug=False, num_devices=N_CORES)
    aq = nc.dram_tensor("aq", [K_AUG, M_LOC], mybir.dt.bfloat16, kind="ExternalInput")
    bs = nc.dram_tensor("bs", [K_AUG, N_TOT], mybir.dt.bfloat16, kind="ExternalInput")
    y = nc.dram_tensor("y", [M_LOC, 1], mybir.dt.float32, kind="ExternalOutput")

    n_mchunks = M_LOC // M_CHUNK  # 16
    n_groups = N_TOT // N_SPAN  # 4
    n_subs = N_SPAN // N_SUB  # 4

    with tile.TileContext(nc) as tc:
        with (
            tc.tile_pool(name="inp", bufs=1) as inp,
            tc.tile_pool(name="acc", bufs=4) as accp,
            tc.tile_pool(name="escratch", bufs=2) as escratch,
            tc.tile_pool(name="out", bufs=4) as outp,
            tc.tile_pool(name="psum", bufs=2, space="PSUM") as psum,
        ):
            a_sb = inp.tile([K_AUG, M_LOC], mybir.dt.bfloat16)
            b_sb = inp.tile([K_AUG, N_TOT], mybir.dt.bfloat16)
            nc.sync.dma_start(out=a_sb, in_=aq[:, :])
            # B in span-sized pieces so the first matmuls only wait on span 0.
            for g in range(n_groups):
                nc.sync.dma_start(
                    out=b_sb[:, g * N_SPAN : (g + 1) * N_SPAN],
                    in_=bs[:, g * N_SPAN : (g + 1) * N_SPAN],
                )

            # PE warm-up: ~5us of dummy matmuls on zeroed scratch while the
            # input DMAs run, so the HAM clock gate opens (1.2 -> 2.4 GHz)
            # before the real matmul stream starts.
            warm = inp.tile([K_AUG, N_SUB], mybir.dt.bfloat16)
            nc.gpsimd.memset(warm, 0.0)
            zw = psum.tile([M_CHUNK, N_SPAN], mybir.dt.float32, tag="z")
            for w in range(12):
                nc.tensor.matmul(
                    zw[:, (w % n_subs) * N_SUB : (w % n_subs + 1) * N_SUB],
                    lhsT=warm[:, 0:M_CHUNK],
                    rhs=warm,
                    start=True,
                    stop=True,
                )

            for mi in range(n_mchunks):
                acc = accp.tile([M_CHUNK, n_groups], mybir.dt.float32)
                lhsT = a_sb[:, mi * M_CHUNK : (mi + 1) * M_CHUNK]
                for g in range(n_groups):
                    z = psum.tile([M_CHUNK, N_SPAN], mybir.dt.float32, tag="z")
                    for s in range(n_subs):
                        n0 = g * N_SPAN + s * N_SUB
                        nc.tensor.matmul(
                            z[:, s * N_SUB : (s + 1) * N_SUB],
                            lhsT=lhsT,
                            rhs=b_sb[:, n0 : n0 + N_SUB],
                            start=True,
                            stop=True,
                        )
                    e = escratch.tile([M_CHUNK, N_SPAN], mybir.dt.float32)
                    nc.scalar.activation(
                        out=e,
                        in_=z,
                        func=mybir.ActivationFunctionType.Exp,
                        accum_out=acc[:, g : g + 1],
                    )
                o = outp.tile([M_CHUNK, 1], mybir.dt.float32)
                nc.vector.reduce_sum(out=o, in_=acc, axis=mybir.AxisListType.X)
                nc.sync.dma_start(
                    out=y[mi * M_CHUNK : (mi + 1) * M_CHUNK, :], in_=o
                )
    nc.compile()
    return nc


def _prep_inputs(x, samples, weights, sigma, length):
    """Host-side O(N+M) folding: build lhsT/rhs matrices [K_AUG, *] in bf16."""
    x = np.asarray(x, dtype=np.float32)
    samples = np.asarray(samples, dtype=np.float32)
    weights = np.asarray(weights, dtype=np.float32)
    sigma = float(np.asarray(sigma))
    length = float(np.asarray(length))

    gamma = 0.5 / (length * length)
    x64 = x.astype(np.float64)
    s64 = samples.astype(np.float64)
    w64 = weights.astype(np.float64)

    # b_n = ln(w_n * sigma^2) - gamma ||s_n||^2 ; c_m = -gamma ||x_m||^2
    with np.errstate(divide="ignore"):
        b = np.log(w64 * (sigma * sigma)) - gamma * np.sum(s64 * s64, axis=1)
    b = np.where(np.isfinite(b), b, -1e30).astype(np.float32)
    c = (-gamma * np.sum(x64 * x64, axis=1)).astype(np.float32)

    xs = (2.0 * gamma) * x  # [M, D] fp32, scaled queries
    xh, xl = _split2(xs)
    sh, sl = _split2(samples)
    bh, bm, bl = _split3(b)
    ch, cm, cl = _split3(c)

    ones_m = np.ones((M_TOT,), dtype=BF16)
    ones_n = np.ones((N_TOT,), dtype=BF16)

    A = np.empty((K_AUG, M_TOT), dtype=BF16)  # lhsT side (queries)
    B = np.empty((K_AUG, N_TOT), dtype=BF16)  # rhs side (samples)
    A[0:D] = xh.T
    A[D : 2 * D] = xl.T
    A[2 * D : 3 * D] = xh.T
    B[0:D] = sh.T
    B[D : 2 * D] = sh.T
    B[2 * D : 3 * D] = sl.T
    # rows 72-74: ones (queries) x b-split (samples)
    A[72], A[73], A[74] = ones_m, ones_m, ones_m
    B[72], B[73], B[74] = bh, bm, bl
    # rows 75-77: c-split (queries) x ones (samples)
    A[75], A[76], A[77] = ch, cm, cl
    B[75], B[76], B[77] = ones_n, ones_n, ones_n

    A = np.ascontiguousarray(A)
    B = np.ascontiguousarray(B)
    return A, B


def kernel(x, samples, weights, sigma, length):
    global LAST_RESULTS
    from concourse.bass_utils import run_bass_kernel_spmd

    if "nc" not in _CACHE:
        _CACHE["nc"] = _build_bass()
    nc = _CACHE["nc"]

    A, B = _prep_inputs(x, samples, weights, sigma, length)
    in_maps = [
        {
            "aq": np.ascontiguousarray(A[:, c * M_LOC : (c + 1) * M_LOC]),
            "bs": B,
        }
        for c in range(N_CORES)
    ]
    trace = bool(os.environ.get("KERNEL_TRACE"))
    res = run_bass_kernel_spmd(
        nc,
        in_maps,
        core_ids=list(range(N_CORES)),
        trace=trace,
    )
    LAST_RESULTS = res
    out = np.concatenate([r["y"] for r in res.results], axis=0)
    return out.astype(np.float32)
